# revision 24
# baseline (speedup 1.0000x reference)
"""Trainium2 Bass kernel: contrastive loss with negative mining.

Math:
    centers  = mean over contiguous chunks of 8 rows               [n/8, d]
    x_pos    = x + 0.5*(center - x)        => |x - x_pos| = 0.5*|x - center|
    sim      = x @ x.T                                             [n, n]
    neg_idx  = argmax_j sim[i, j] excluding j in i's group-of-4
    d_ap     = mean_d |x - x_pos|,  d_an = mean_d |x - x_neg|
    loss     = sum( (1/8) * d_ap / (d_an + 1e-7) )

Distribution: data-parallel over rows, 8 NeuronCores, 1024 rows each.
Every core receives the full x (and an fp8/bf16 x.T) in its own DRAM, so no
collectives are needed; per-row losses are returned and summed on host.

Per core:
  - sim rows are fp8e4m3 DoubleRow matmuls (stationary = xT slice of this
    core's rows, moving = full xT) in 512-wide column strips, f32 PSUM
    accumulation, evacuated to SBUF as bf16 by ScalarE.
  - Column strips are processed in a per-core ROTATED order (the moving
    operand is pre-rotated on the host), so the "diagonal" strips that
    contain this core's own columns are always strips t < ND.  Only those
    need the group-of-4 exclusion, so only they use the expensive DVE
    max/max_index top-8; the remaining strips use a cheap top-1:
    reduce_max + (is_ge ? idx-BIGI : 0) + reduce_min.
  - A row's excluded group spans at most 4 of a diagonal strip's top-8,
    so the best valid candidate always survives.  Candidates are masked
    with compares against per-partition group bounds (input data), then
    reduced to the argmax index.
  - x_neg rows are gathered from DRAM with a GPSIMD indirect DMA.
  - d_ap uses y = (I - blockdiag(ones(8,8)/8)) @ x_tile (bf16 matmul,
    emitted after the sim so it overlaps the tail) with ScalarE
    Abs+accumulate; d_an uses a DVE subtract + ScalarE Abs+accumulate.
"""

import math

import ml_dtypes
import numpy as np

import concourse.bass as bass
import concourse.mybir as mybir
import concourse.tile as tile
from concourse import bacc
from concourse.bass import IndirectOffsetOnAxis
from concourse.bass_utils import run_bass_kernel_spmd

BF16 = mybir.dt.bfloat16
F32 = mybir.dt.float32
U32 = mybir.dt.uint32
ALU = mybir.AluOpType
ACTF = mybir.ActivationFunctionType
AXX = mybir.AxisListType.X

P = 128         # partitions / row-tile height
JS = 512        # similarity column-strip width
CHUNK = 8       # rows averaged per center
GROUP = 4       # negative-mining exclusion window
WEIGHT = 1.0 / 8
EPS = 1e-7
NEG_BIG = -1e30
BIGI = 65536.0  # index bias for the top-1 min-index trick


class Cfg:
    def __init__(self, n=8192, d=2048, cores=8, fp8=True):
        self.n, self.d, self.cores, self.fp8 = n, d, cores, fp8
        self.r = n // cores            # rows per core
        self.it = self.r // P          # i-tiles per core
        self.nj = n // JS              # column strips
        self.kb = d // P               # contraction blocks
        self.cw = min(d, JS)           # d-chunk width for the d_ap matmul
        self.ch = d // self.cw         # number of d-chunks
        self.nd = max(1, self.r // JS)  # diagonal strips (hold own columns)
        self.no = self.nj - self.nd    # off-diagonal strips
        self.ncand = self.nd * 8 + self.no
        assert n % (cores * P) == 0 and d % P == 0 and n % JS == 0
        assert d % self.cw == 0 and JS % self.r == 0 or self.r % JS == 0

    def s0(self, c):                   # global strip of core c's first row
        return (c * self.r) // JS

    def strip_of(self, c, t):          # global strip processed at slot t
        return (self.s0(c) + t) % self.nj


def _body(tc: tile.TileContext, cfg: Cfg, io: dict):
    nc = tc.nc
    ctxpools = {}

    def pool(name, bufs, space="SBUF"):
        if name not in ctxpools:
            ctxpools[name] = tc.alloc_tile_pool(name=name, bufs=bufs, space=space)
        return ctxpools[name]

    sim_dt = mybir.dt.float8e4 if cfg.fp8 else BF16

    # resident stationary xT slice: [128, KB*R], k-block major
    xs_sb = pool("xs", 1).tile([P, cfg.kb * cfg.r], sim_dt, name="xs_sb")
    nc.sync.dma_start(
        out=xs_sb[:].rearrange("p (a r) -> p a r", a=cfg.kb),
        in_=io["xs"][:, :].rearrange("(a p) r -> p a r", p=P),
    )

    consts = pool("consts", 1)
    idxmb_sb = consts.tile_from(io["idxmb"])                 # [128,JS] f32
    m2b_sb = consts.tile_from(io["m2b"])                     # [128,128] bf16
    g0_sb = consts.tile_from(io["g0f"])                      # [128,IT] f32
    g3_sb = consts.tile_from(io["g3f"])                      # [128,IT] f32
    offd_sb = consts.tile_from(io["offd"])                   # [128,ND*8] f32
    offb_sb = consts.tile_from(io["offb"])                   # [128,NO] f32

    psum = pool("ps", 8, space="PSUM")
    small = pool("small", 1)
    sap = small.tile([P, cfg.it * cfg.ch], F32, name="sap")    # sum|y| per chunk
    san = small.tile([P, cfg.it], F32, name="san")             # sum|x-xneg|
    idxall = small.tile([P, cfg.it], U32, name="idxall")       # neg indices
    cv_sb = small.tile([P, cfg.it * cfg.nd * 8], BF16, name="cv_sb")
    ci_sb = small.tile([P, cfg.it * cfg.nd * 8], U32, name="ci_sb")
    # per-i-tile candidate values [diag top8s | off-diag top1s] and raw
    # (idx - BIGI) min-indices of the off-diag strips
    v30 = [small.tile([P, cfg.ncand], F32, name=f"v30_{it}", tag=f"v30_{it}")
           for it in range(cfg.it)]
    sxr = [small.tile([P, cfg.no], F32, name=f"sxr_{it}", tag=f"sxr_{it}")
           for it in range(cfg.it)]

    # ---- Phase B: sim strips; top-8 on diagonal strips, top-1 elsewhere ----
    xmp = pool("xm", 2)
    evac = pool("evac", 4)
    pickp = pool("pick", 4)
    nd8 = cfg.nd * 8
    for t in range(cfg.nj):
        xm_sb = xmp.tile([P, cfg.kb * JS], sim_dt, name="xm_sb")
        nc.sync.dma_start(
            out=xm_sb[:].rearrange("p (a b) -> p a b", a=cfg.kb),
            in_=io["xm"][:, t * JS:(t + 1) * JS].rearrange("(a p) b -> p a b", p=P),
        )
        xs3 = xs_sb[:].rearrange("p (a r) -> p a r", a=cfg.kb)
        xm3 = xm_sb[:].rearrange("p (a b) -> p a b", a=cfg.kb)
        for it in range(cfg.it):
            ps_s = psum.tile([P, JS], F32, name="ps_s", tag="ps")
            if cfg.fp8:
                for k in range(0, cfg.kb, 2):
                    nc.tensor.matmul(
                        out=ps_s[:],
                        lhsT=xs3[:, k:k + 2, it * P:(it + 1) * P],
                        rhs=xm3[:, k:k + 2, :],
                        start=(k == 0), stop=(k == cfg.kb - 2),
                        perf_mode=mybir.MatmulPerfMode.DoubleRow,
                    )
            else:
                for k in range(cfg.kb):
                    nc.tensor.matmul(
                        out=ps_s[:],
                        lhsT=xs_sb[:, k * cfg.r + it * P: k * cfg.r + (it + 1) * P],
                        rhs=xm_sb[:, k * JS:(k + 1) * JS],
                        start=(k == 0), stop=(k == cfg.kb - 1),
                    )
            sstrip = evac.tile([P, JS], BF16, name="sstrip")
            nc.scalar.copy(out=sstrip[:], in_=ps_s[:])
            if t < cfg.nd:
                q0 = (it * cfg.nd + t) * 8
                nc.vector.max(out=cv_sb[:, q0:q0 + 8], in_=sstrip[:])
                nc.vector.max_index(
                    out=ci_sb[:, q0:q0 + 8],
                    in_max=cv_sb[:, q0:q0 + 8],
                    in_values=sstrip[:],
                )
            else:
                q = nd8 + (t - cfg.nd)
                sv = v30[it][:, q:q + 1]
                nc.vector.tensor_reduce(out=sv, in_=sstrip[:], axis=AXX, op=ALU.max)
                pick = pickp.tile([P, JS], F32, name="pick")
                # pick = (sstrip >= max) * (idx - BIGI); zeros elsewhere
                nc.vector.scalar_tensor_tensor(
                    out=pick[:], in0=sstrip[:], scalar=sv, in1=idxmb_sb[:],
                    op0=ALU.is_ge, op1=ALU.mult,
                )
                nc.vector.tensor_reduce(
                    out=sxr[it][:, t - cfg.nd:t - cfg.nd + 1], in_=pick[:],
                    axis=AXX, op=ALU.min,
                )

    # ---- Phase A (emitted after B so PE starts on sim immediately):
    #      d_ap = sum_d |y|,  y = M2 @ x_tile  (bf16 matmuls, overlap tail) ----
    xrbp = pool("xrb", 2)
    yabs = pool("yabs", 2)
    for it in range(cfg.it):
        xrb_t = xrbp.tile([P, cfg.d], BF16, name="xrb_t")
        nc.sync.dma_start(out=xrb_t[:], in_=io["xrb"][it * P:(it + 1) * P, :])
        for c in range(cfg.ch):
            ps_y = psum.tile([P, cfg.cw], F32, name="ps_y", tag="ps")
            nc.tensor.matmul(
                out=ps_y[:], lhsT=m2b_sb[:],
                rhs=xrb_t[:, c * cfg.cw:(c + 1) * cfg.cw],
                start=True, stop=True,
            )
            y_sc = yabs.tile([P, cfg.cw], F32, name="y_sc")
            nc.scalar.activation(
                out=y_sc[:], in_=ps_y[:], func=ACTF.Abs,
                accum_out=sap[:, it * cfg.ch + c: it * cfg.ch + c + 1],
            )

    # resident x rows (f32) for d_an
    xr_sb = pool("xr", 1).tile([P, cfg.it * cfg.d], F32, name="xr_sb")
    nc.sync.dma_start(
        out=xr_sb[:].rearrange("p (a d) -> p a d", a=cfg.it),
        in_=io["xr"][:, :].rearrange("(a p) d -> p a d", p=P),
    )

    # ---- Phase C: combine candidates, gather x_neg, d_an, loss ----
    comb = pool("comb", 2)
    xneg_p = pool("xneg", 2)
    diff_p = pool("diff", 2)
    dabs_p = pool("dabs", 2)
    for it in range(cfg.it):
        # diagonal candidates: global idx + group-exclusion mask
        cvf = comb.tile([P, nd8], F32, name="cvf")
        nc.vector.tensor_copy(out=cvf[:], in_=cv_sb[:, it * nd8:(it + 1) * nd8])
        cif = comb.tile([P, nd8], F32, name="cif")
        nc.vector.tensor_copy(out=cif[:], in_=ci_sb[:, it * nd8:(it + 1) * nd8])
        gidx = comb.tile([P, cfg.ncand], F32, name="gidx")
        nc.vector.tensor_tensor(
            out=gidx[:, 0:nd8], in0=cif[:], in1=offd_sb[:], op=ALU.add)
        b1 = comb.tile([P, nd8], F32, name="b1")
        nc.vector.tensor_scalar(
            out=b1[:], in0=gidx[:, 0:nd8], scalar1=g0_sb[:, it:it + 1],
            scalar2=None, op0=ALU.is_ge)
        b2 = comb.tile([P, nd8], F32, name="b2")
        nc.vector.tensor_scalar(
            out=b2[:], in0=gidx[:, 0:nd8], scalar1=g3_sb[:, it:it + 1],
            scalar2=None, op0=ALU.is_le)
        msk = comb.tile([P, nd8], F32, name="msk")
        nc.vector.scalar_tensor_tensor(
            out=msk[:], in0=b1[:], scalar=NEG_BIG, in1=b2[:],
            op0=ALU.mult, op1=ALU.mult)
        nc.vector.tensor_tensor(
            out=v30[it][:, 0:nd8], in0=cvf[:], in1=msk[:], op=ALU.add)
        # off-diag: global idx = raw + (BIGI + strip_base)
        nc.vector.tensor_tensor(
            out=gidx[:, nd8:], in0=sxr[it][:], in1=offb_sb[:], op=ALU.add)
        # argmax over all candidates
        mx = comb.tile([P, 1], F32, name="mx")
        nc.vector.tensor_reduce(out=mx[:], in_=v30[it][:], axis=AXX, op=ALU.max)
        sel = comb.tile([P, cfg.ncand], F32, name="sel")
        nc.vector.tensor_scalar(
            out=sel[:], in0=v30[it][:], scalar1=mx[:], scalar2=None, op0=ALU.is_ge)
        pk = comb.tile([P, cfg.ncand], F32, name="pk")
        nc.vector.tensor_tensor(out=pk[:], in0=sel[:], in1=gidx[:], op=ALU.mult)
        idxf = comb.tile([P, 1], F32, name="idxf")
        nc.vector.tensor_reduce(out=idxf[:], in_=pk[:], axis=AXX, op=ALU.max)
        nc.vector.tensor_copy(out=idxall[:, it:it + 1], in_=idxf[:])

        xneg = xneg_p.tile([P, cfg.d], F32, name="xneg")
        nc.gpsimd.indirect_dma_start(
            out=xneg[:], out_offset=None,
            in_=io["xf"][:, :],
            in_offset=IndirectOffsetOnAxis(ap=idxall[:, it:it + 1], axis=0),
        )
        diff = diff_p.tile([P, cfg.d], F32, name="diff")
        nc.vector.tensor_tensor(
            out=diff[:], in0=xr_sb[:, it * cfg.d:(it + 1) * cfg.d], in1=xneg[:],
            op=ALU.subtract,
        )
        dabs = dabs_p.tile([P, cfg.d], BF16, name="dabs")
        nc.scalar.activation(
            out=dabs[:], in_=diff[:], func=ACTF.Abs,
            accum_out=san[:, it:it + 1],
        )

    # ---- Final: per-row loss ----
    fin = pool("fin", 1)
    sap8 = fin.tile([P, cfg.it], F32, name="sap8")
    sap3 = sap[:].rearrange("p (a b) -> p a b", a=cfg.it)
    nc.vector.tensor_reduce(out=sap8[:], in_=sap3, axis=AXX, op=ALU.add)
    t1 = fin.tile([P, cfg.it], F32, name="t1")
    nc.vector.tensor_scalar(
        out=t1[:], in0=san[:], scalar1=1.0 / cfg.d, scalar2=EPS,
        op0=ALU.mult, op1=ALU.add,
    )
    rec = fin.tile([P, cfg.it], F32, name="rec")
    nc.vector.reciprocal(out=rec[:], in_=t1[:])
    t2 = fin.tile([P, cfg.it], F32, name="t2")
    nc.vector.tensor_tensor(out=t2[:], in0=sap8[:], in1=rec[:], op=ALU.mult)
    lossv = fin.tile([P, cfg.it], F32, name="lossv")
    nc.vector.tensor_scalar(
        out=lossv[:], in0=t2[:], scalar1=0.5 * WEIGHT / cfg.d, scalar2=None,
        op0=ALU.mult,
    )
    nc.sync.dma_start(out=io["loss_part"][:, :], in_=lossv[:])
    nc.sync.dma_start(out=io["nidx"][:, :], in_=idxall[:])

    for p in reversed(list(ctxpools.values())):
        p.release()


def build(cfg: Cfg) -> bass.Bass:
    nc = bacc.Bacc("TRN2", target_bir_lowering=False, debug=False)
    sim_dt = mybir.dt.float8e4 if cfg.fp8 else BF16
    io = {
        "xm": nc.dram_tensor("xm", [cfg.d, cfg.n], sim_dt, kind="ExternalInput").ap(),
        "xs": nc.dram_tensor("xs", [cfg.d, cfg.r], sim_dt, kind="ExternalInput").ap(),
        "xr": nc.dram_tensor("xr", [cfg.r, cfg.d], F32, kind="ExternalInput").ap(),
        "xrb": nc.dram_tensor("xrb", [cfg.r, cfg.d], BF16, kind="ExternalInput").ap(),
        "xf": nc.dram_tensor("xf", [cfg.n, cfg.d], F32, kind="ExternalInput").ap(),
        "m2b": nc.dram_tensor("m2b", [P, P], BF16, kind="ExternalInput").ap(),
        "idxmb": nc.dram_tensor("idxmb", [P, JS], F32, kind="ExternalInput").ap(),
        "offd": nc.dram_tensor("offd", [P, cfg.nd * 8], F32, kind="ExternalInput").ap(),
        "offb": nc.dram_tensor("offb", [P, cfg.no], F32, kind="ExternalInput").ap(),
        "g0f": nc.dram_tensor("g0f", [P, cfg.it], F32, kind="ExternalInput").ap(),
        "g3f": nc.dram_tensor("g3f", [P, cfg.it], F32, kind="ExternalInput").ap(),
        "loss_part": nc.dram_tensor("loss_part", [P, cfg.it], F32, kind="ExternalOutput").ap(),
        "nidx": nc.dram_tensor("nidx", [P, cfg.it], U32, kind="ExternalOutput").ap(),
    }
    with tile.TileContext(nc) as tc:
        _body(tc, cfg, io)
    nc.compile()
    return nc


def make_in_maps(cfg: Cfg, x: np.ndarray) -> list[dict]:
    x = np.ascontiguousarray(x, dtype=np.float32)
    sim_np = ml_dtypes.float8_e4m3 if cfg.fp8 else ml_dtypes.bfloat16
    xt_q = np.ascontiguousarray(x.T.astype(sim_np))
    x_bf = x.astype(ml_dtypes.bfloat16)

    m2 = np.eye(P, dtype=np.float32)
    for c in range(P // CHUNK):
        m2[c * CHUNK:(c + 1) * CHUNK, c * CHUNK:(c + 1) * CHUNK] -= 1.0 / CHUNK
    m2b = m2.astype(ml_dtypes.bfloat16)

    idxmb = np.broadcast_to(
        np.arange(JS, dtype=np.float32) - BIGI, (P, JS)).copy()

    pvec = np.arange(P, dtype=np.float32)
    in_maps = []
    for c in range(cfg.cores):
        cut = cfg.s0(c) * JS
        xm_rot = np.ascontiguousarray(
            np.concatenate([xt_q[:, cut:], xt_q[:, :cut]], axis=1))
        offd = np.zeros((P, cfg.nd * 8), dtype=np.float32)
        for t in range(cfg.nd):
            offd[:, t * 8:(t + 1) * 8] = cfg.strip_of(c, t) * JS
        offb = np.zeros((P, cfg.no), dtype=np.float32)
        for t in range(cfg.nd, cfg.nj):
            offb[:, t - cfg.nd] = BIGI + cfg.strip_of(c, t) * JS
        g0 = np.zeros((P, cfg.it), dtype=np.float32)
        for it in range(cfg.it):
            g0[:, it] = c * cfg.r + it * P + (pvec // GROUP) * GROUP
        in_maps.append({
            "xm": xm_rot,
            "xs": np.ascontiguousarray(xt_q[:, c * cfg.r:(c + 1) * cfg.r]),
            "xr": np.ascontiguousarray(x[c * cfg.r:(c + 1) * cfg.r]),
            "xrb": np.ascontiguousarray(x_bf[c * cfg.r:(c + 1) * cfg.r]),
            "xf": x,
            "m2b": m2b,
            "idxmb": idxmb,
            "offd": offd,
            "offb": offb,
            "g0f": g0,
            "g3f": g0 + (GROUP - 1),
        })
    return in_maps


def reduce_outputs(cfg: Cfg, results: list[dict]) -> np.ndarray:
    total = 0.0
    for res in results:
        total += float(res["loss_part"].astype(np.float64).sum())
    return np.float32(total)


def run(cfg: Cfg, x: np.ndarray, trace: bool = False):
    nc = build(cfg)
    in_maps = make_in_maps(cfg, x)
    out = run_bass_kernel_spmd(nc, in_maps, list(range(cfg.cores)), trace=trace)
    return out


def kernel(x: np.ndarray) -> np.ndarray:
    cfg = Cfg(n=8192, d=2048, cores=8)
    out = run(cfg, x)
    return reduce_outputs(cfg, out.results)
